# revision 51
# baseline (speedup 1.0000x reference)
"""TRN2 Bass kernel for nn_Attention_75935021793702.

Dense transformer attention block:
    qkv = x @ Wqkv ; q,k = RoPE(q,k,pos) ; y = softmax(causal(q k^T / sqrt(dk))) v ; out = y @ Wo

Sharding: hybrid 2 (batch) x 4 (head-group) over 8 cores.  Each core handles
one batch and 4 heads: its slice of the QKV projection (columns of Wqkv), the
attention for its 4 heads, and a partial output projection (rows of Wo).  The
host sums 4 partials per batch.

All 16-bit operands are fp16 (better mantissa than bf16 at these magnitudes;
matmuls run at full PE rate).  PSUM accumulation is fp32.

Device dataflow (per core), fully fused pipeline over 4 token chunks j:
  phaseA(j): stream x^T chunk -> q^T,k^T (feature-major + RoPE on DVE) and v
             (token-major); q/k/v stay RESIDENT in SBUF (no DRAM spill).
  attn(h,j): S^T = k^T.T q^T on PE; exp on ACT (scale=1/sqrt(dk)) into an
             es strip [128, L, 512]; causal boundary via one shared
             [128,128] multiplicative 0/1 triangle on a 128-col window
             post-exp (gpsimd); A@V accumulates O^T on PE.  Softmax
             denominator: fp16 halving-tree partial sum on DVE down to <=4
             slots, then accumulating 128-wide all-ones matmuls on PE whose
             output replicates the cross-partition sum into every partition
             (no broadcast needed); on the last chunk the whole rowsum runs
             as per-tile all-ones matmuls on the then-idle PE/pap bank.
             reciprocal + O^T scale on DVE.
  proj(j):   out[tq,:] += sum_h O_h^T.T @ Wo_h, PSUM -> SBUF -> DRAM.
  phaseA(j+1) and proj(j-1) are emitted as FILLER work between (and inside)
  the attention instances of chunk j so the PE queue never drains.
"""

import sys

sys.path.insert(0, "/opt/trn_rl_repo")

import numpy as np
import concourse.bass as bass
import concourse.mybir as mybir
import concourse.tile as tile
from concourse import bacc
from concourse import bass_isa
from concourse.bass_utils import run_bass_kernel_spmd

F32 = mybir.dt.float32
F16 = mybir.dt.float16
EXP = mybir.ActivationFunctionType.Exp

HDT = F16             # dtype of all 16-bit matmul operands
NEG = -1.0e9          # additive causal-mask value (pre-exp)

B, T, D, H = 2, 2048, 2048, 16
DK = D // H                       # 128
THETA = 10000.0
NCORES = 8
HG = 4                            # head groups (cores per batch)
HPC = H // HG                     # heads per core = 4
DL = HPC * DK                     # local width = 512
TCH = 512                         # token chunk (matmul moving dim)
NCHB = T // TCH                   # 4 chunks per batch
KT = D // 128                     # 16 contraction tiles
NTT = T // 128                    # 16 tk tiles
SCALE = 1.0 / float(np.sqrt(np.float32(DK)))

_cache = {}


def _mm(nc, out, lhsT, rhs, start, stop):
    nc.tensor.matmul(out, lhsT, rhs, start=start, stop=stop)


def _build():
    nc = bacc.Bacc("TRN2", target_bir_lowering=False, debug=False)

    xt_d = nc.dram_tensor("xt", [D, T], HDT, kind="ExternalInput").ap()
    wqkv_d = nc.dram_tensor("wqkv", [D, 3 * DL], HDT, kind="ExternalInput").ap()
    wo_d = nc.dram_tensor("wo", [DL, D], HDT, kind="ExternalInput").ap()
    cs2_d = nc.dram_tensor("cs2", [DK, T], F32, kind="ExternalInput").ap()
    sn2_d = nc.dram_tensor("sn2", [DK, T], F32, kind="ExternalInput").ap()
    mp_d = nc.dram_tensor("mp", [128, 128], HDT, kind="ExternalInput").ap()
    out_d = nc.dram_tensor("out", [T, D], HDT, kind="ExternalOutput").ap()

    with tile.TileContext(nc) as tc:
        with (
            tc.tile_pool(name="const", bufs=1) as pc,
            tc.tile_pool(name="pax", bufs=18) as pax,      # x^T stream
            tc.tile_pool(name="pq", bufs=8) as pq,         # q^T chunk tiles
            tc.tile_pool(name="pat", bufs=3) as pat,       # RoPE temporaries
            tc.tile_pool(name="pes", bufs=2) as pes,       # exp(S) strips
            tc.tile_pool(name="prb", bufs=3) as prb,       # rowsum bcast/recip
            tc.tile_pool(name="posb", bufs=8) as posb,     # normalized O^T
            tc.tile_pool(name="pcs", bufs=3) as pcs,       # out staging
            tc.tile_pool(name="pap", bufs=2, space="PSUM") as pap,   # phase A
            tc.tile_pool(name="pbs", bufs=2, space="PSUM") as pbs,   # S
            tc.tile_pool(name="pbp", bufs=2, space="PSUM") as pbp,   # O acc
            tc.tile_pool(name="pcp", bufs=2, space="PSUM") as pcp,   # proj
        ):
            # ---------------- constants / resident tensors ----------------
            wq = pc.tile([128, KT, 3 * DL], HDT)
            cs2 = pc.tile([128, T], F32)
            sn2 = pc.tile([128, T], F32)
            mp = pc.tile([128, 128], HDT)   # 0/1 upper triangle (tk <= tq)
            wo = pc.tile([128, HPC, D], HDT)
            k_res = pc.tile([128, HPC, T], HDT)     # k^T feature-major
            v_res = pc.tile([128, NTT, DL], HDT)    # v token-major
            ones = pc.tile([128, 128], HDT)         # rowsum+broadcast matmul
            # memset on gpsimd: its queue is empty at t=0, unlike the DVE
            # which sits behind the framework's preamble const loads
            nc.gpsimd.memset(ones[:, :], 1.0)
            # dependency-free junk matmuls spanning the whole DMA lead-in:
            # trips the HAM activity window early AND keeps it warm (it
            # re-throttles after ~3.4us idle), so the PE clock is already
            # 2.4 GHz when the first real matmul issues at ~13us
            # warm lives in the proj PSUM pool, idle until j=1: junk can keep
            # firing between chunk-0 ki-rounds without touching the pools the
            # real chunk-0 groups rotate through
            warm = pcp.tile([128, TCH], F32, tag="pso", name="warm")
            for w in range(140):
                _mm(nc, warm[0:32, 0:32], ones[:, 0:32], ones[:, 0:32],
                    w == 0, w == 139)

            def junk(n):
                for w in range(n):
                    _mm(nc, warm[0:32, 0:32], ones[:, 0:32], ones[:, 0:32],
                        w == 0, w == n - 1)

            xts = {}     # (j, ki) -> xt tile
            q_sb = {}    # (j, h) -> q^T tile
            osbs = {}    # (h, j) -> normalized O^T tile

            def dma_chunk(j):
                tc0 = j * TCH
                for ki in range(KT):
                    if j == 0:
                        # q/k columns only: the first six matmul groups gate
                        # on these, the v columns stream afterwards
                        nc.sync.dma_start(
                            wq[:, ki, 0 : 2 * DL],
                            wqkv_d[128 * ki : 128 * ki + 128, 0 : 2 * DL],
                        )
                    xt = pax.tile([128, TCH], HDT, tag="xt", name=f"xt_{j}_{ki}")
                    nc.sync.dma_start(
                        xt[:, :], xt_d[128 * ki : 128 * ki + 128, tc0 : tc0 + TCH]
                    )
                    xts[(j, ki)] = xt
                    if j == 0 and ki == 1:
                        # RoPE/mask constants must beat the first psqk group
                        nc.sync.dma_start(cs2[:, :], cs2_d[:, :])
                        nc.sync.dma_start(sn2[:, :], sn2_d[:, :])
                        nc.sync.dma_start(mp[:, :], mp_d[:, :])
                if j == 0:
                    for ki in range(KT):
                        nc.sync.dma_start(
                            wq[:, ki, 2 * DL : 3 * DL],
                            wqkv_d[128 * ki : 128 * ki + 128, 2 * DL : 3 * DL],
                        )

            def dma_wo():
                for dt in range(HPC):
                    nc.sync.dma_start(wo[:, dt, :], wo_d[128 * dt : 128 * dt + 128, :])

            def mm_qk(j, nt, psqk, ki):
                _mm(nc, psqk[:, :], wq[:, ki, 128 * nt : 128 * nt + 128],
                    xts[(j, ki)][:, :], ki == 0, ki == KT - 1)

            def mm_v(j, tt, psv, ki):
                _mm(nc, psv[:, :], xts[(j, ki)][:, 128 * tt : 128 * tt + 128],
                    wq[:, ki, 2 * DL : 3 * DL], ki == 0, ki == KT - 1)

            def fin_qk(j, nt, psqk):
                # Free the PSUM bank immediately with a fast ACT copy into the
                # destination tile, then RoPE in-place on DVE (the DVE is not
                # on the PE's PSUM-rotation critical path this way).
                tc0 = j * TCH
                if nt < 4:
                    qt = pq.tile([128, TCH], HDT, tag="qt", name=f"qt_{j}_{nt}")
                    q_sb[(j, nt)] = qt
                    dst = lambda p0, p1: qt[p0:p1, :]
                else:
                    kh = nt - 4
                    dst = lambda p0, p1: k_res[p0:p1, kh, tc0 : tc0 + TCH]
                nc.scalar.copy(dst(0, 128), psqk[:, :])
                # sn2 halves are host-swapped so both SBUF inputs share the
                # same base partition (BIR verifier constraint)
                t2 = pat.tile([128, TCH], F32, tag="t2", name=f"t2_{j}_{nt}")
                nc.vector.tensor_mul(t2[0:64, :], dst(64, 128), sn2[64:128, tc0 : tc0 + TCH])
                nc.vector.tensor_mul(t2[64:128, :], dst(0, 64), sn2[0:64, tc0 : tc0 + TCH])
                t1 = pat.tile([128, TCH], F32, tag="t1", name=f"t1_{j}_{nt}")
                nc.vector.tensor_mul(t1[:, :], dst(0, 128), cs2[:, tc0 : tc0 + TCH])
                nc.vector.tensor_add(dst(0, 128), t1[:, :], t2[:, :])

            def fin_v(j, tt, psv):
                nc.scalar.copy(v_res[:, 4 * j + tt, :], psv[:, :])

            def qk_thunk(j, nt):
                def th():
                    psqk = pap.tile([128, TCH], F32, tag="ps", name=f"psqk_{j}_{nt}")
                    for ki in range(KT):
                        mm_qk(j, nt, psqk, ki)
                    fin_qk(j, nt, psqk)
                return th

            def v_thunk(j, tt):
                def th():
                    psv = pap.tile([128, DL], F32, tag="ps", name=f"psv_{j}_{tt}")
                    for ki in range(KT):
                        mm_v(j, tt, psv, ki)
                    fin_v(j, tt, psv)
                return th

            def phaseA_thunks(j):
                ths = [lambda j=j: dma_chunk(j)]
                ths += [qk_thunk(j, nt) for nt in range(8)]
                ths += [v_thunk(j, tt) for tt in range(4)]
                return ths

            def phaseA_chunk0():
                # Chunk 0 runs while weights/x still stream from HBM: issue
                # matmuls ki-major across 6 concurrently-open PSUM banks
                # (borrowing the idle attention pools) so the PE consumes
                # each arriving DMA tile for 6 groups at once instead of
                # stalling per-group.
                groups_a = [("q", 0), ("k", 4), ("q", 1), ("k", 5), ("q", 2), ("k", 6)]
                groups_b = [("q", 3), ("k", 7), ("v", 0), ("v", 1), ("v", 2), ("v", 3)]
                pools = [pap, pap, pbs, pbs, pbp, pbp]
                tags = ["ps", "ps", "ps_s", "ps_s", "ps_o", "ps_o"]
                for gi, grp in enumerate([groups_a, groups_b]):
                    ps = []
                    for g, (kind, idx) in enumerate(grp):
                        ps.append(pools[g].tile([128, TCH], F32, tag=tags[g],
                                                name=f"ps0_{gi}_{g}"))
                    for ki in range(KT):
                        for g, (kind, idx) in enumerate(grp):
                            if kind == "v":
                                mm_v(0, idx, ps[g], ki)
                            else:
                                mm_qk(0, idx, ps[g], ki)
                        # fill DMA-wait bubbles in the ramp and keep the
                        # HAM activity window tripped
                        junk(2)
                    for g, (kind, idx) in enumerate(grp):
                        if kind == "v":
                            fin_v(0, idx, ps[g])
                        else:
                            fin_qk(0, idx, ps[g])

            def proj_thunks(j):
                # out[tq,:] += sum_h O_h[tq,dl] @ Wo_h[dl,:] for chunk j
                ths = []
                for a in range(TCH // 128):
                    for ec in range(D // TCH):
                        def th(a=a, ec=ec, j=j):
                            trow = TCH * j + 128 * a
                            pso = pcp.tile([128, TCH], F32, tag="pso",
                                           name=f"psoc_{j}_{a}_{ec}")
                            for h in range(HPC):
                                _mm(nc, pso[:, :],
                                    osbs[(h, j)][:, 128 * a : 128 * a + 128],
                                    wo[:, h, TCH * ec : TCH * ec + TCH],
                                    h == 0, h == HPC - 1)
                            outsb = pcs.tile([128, TCH], HDT, tag="outsb",
                                             name=f"outsb_{j}_{a}_{ec}")
                            # DVE while attention is live (the scalar engine
                            # is exp-critical there); the last chunk's
                            # projection runs in the exp-free tail, where
                            # alternating engines doubles copy throughput
                            if j == NCHB - 1 and (a + ec) % 2 == 0:
                                nc.scalar.copy(outsb[:, :], pso[:, :])
                            else:
                                nc.vector.tensor_copy(outsb[:, :], pso[:, :])
                            nc.sync.dma_start(
                                out_d[trow : trow + 128, TCH * ec : TCH * ec + TCH],
                                outsb[:, :],
                            )
                        ths.append(th)
                return ths

            pending = {"tail": [], "fin": None}
            filler = []

            def flush_pending():
                for fn in pending["tail"]:
                    fn()
                pending["tail"] = []
                if pending["fin"] is not None:
                    pending["fin"]()
                    pending["fin"] = None

            def pop_filler(n):
                for _ in range(min(n, len(filler))):
                    filler.pop(0)()

            def attention(h, j):
                qt = q_sb[(j, h)]
                L = 4 * j + 4           # live tk tiles 0..L-1
                # Last chunk: no phase-A fillers exist, the PE has slack and
                # the pap PSUM pool is idle -> do the softmax denominator as
                # per-tile accumulating all-ones matmuls instead of the DVE
                # tree (the DVE is the bottleneck there).
                pe_rowsum = j == NCHB - 1
                ps_o = pbp.tile([128, TCH], F32, tag="ps_o", name=f"pso_{h}_{j}")
                ps_r = (pap.tile([128, TCH], F32, tag="ps", name=f"psr_{h}_{j}")
                        if pe_rowsum else None)
                es = pes.tile([128, NTT, TCH], HDT, tag="es", name=f"es_{h}_{j}")
                SKEW = 2

                def consume(i, idx):
                    st = idx == 0
                    sp = idx == L - 1
                    sl = 128 * (i - 4 * j) if i >= 4 * j else 0
                    _mm(nc, ps_o[:, sl:TCH],
                        v_res[:, i, 128 * h : 128 * h + 128],
                        es[:, i, sl:TCH], st, sp)
                    if pe_rowsum:
                        _mm(nc, ps_r[:, sl:TCH], ones[:, :], es[:, i, sl:TCH], st, sp)

                prev_tail = pending["tail"]
                prev_fin = pending["fin"]
                fin_done = [prev_fin is None]

                for i in range(L):
                    diag = i >= 4 * j
                    sl = 128 * (i - 4 * j) if diag else 0
                    ps_s = pbs.tile([128, TCH], F32, tag="ps_s", name=f"pss_{h}_{j}_{i}")
                    _mm(nc, ps_s[:, sl:TCH],
                        k_res[:, h, 128 * i : 128 * i + 128],
                        qt[:, sl:TCH], True, True)
                    if diag and sl > 0 and not pe_rowsum:
                        nc.gpsimd.memset(es[:, i, 0:sl], 0.0)
                    nc.scalar.activation(es[:, i, sl:TCH], ps_s[:, sl:TCH], EXP, scale=SCALE)
                    if diag:
                        # causal boundary: multiplicative 0/1 triangle on the
                        # 128-col window, post-exp, on the otherwise-idle
                        # gpsimd engine (it cannot touch PSUM, es is SBUF)
                        nc.gpsimd.tensor_mul(
                            es[:, i, sl : sl + 128], es[:, i, sl : sl + 128], mp[:, :]
                        )
                    # drain the previous instance's deferred work, one step per
                    # S-matmul, so the PE never waits on freshly issued exps
                    if prev_tail:
                        prev_tail.pop(0)()
                    elif not fin_done[0]:
                        prev_fin()
                        fin_done[0] = True
                    elif i % 2 == 1:
                        pop_filler(1)
                    if i >= SKEW:
                        consume(i - SKEW, i - SKEW)
                while prev_tail:
                    prev_tail.pop(0)()
                if not fin_done[0]:
                    prev_fin()
                    fin_done[0] = True

                def finalize():
                    if pe_rowsum:
                        ps_rr = ps_r
                    else:
                        # denominator: in-place fp16 halving-tree sum down to
                        # <=4 slots on DVE, then accumulating 128-wide
                        # all-ones matmuls on PE that sum across partitions
                        # AND replicate the result into every output
                        # partition (no broadcast needed)
                        lc = L
                        while lc > 4:
                            half = lc // 2
                            nc.vector.tensor_add(
                                es[:, 0:half, :], es[:, 0:half, :],
                                es[:, lc - half : lc, :]
                            )
                            lc -= half
                        ps_rr = pbs.tile([128, TCH], F32, tag="ps_s", name=f"psr_{h}_{j}")
                        for g in range(lc):
                            _mm(nc, ps_rr[:, :], ones[:, :], es[:, g, :],
                                g == 0, g == lc - 1)
                    rinv = prb.tile([128, TCH], F32, tag="rinv", name=f"rinv_{h}_{j}")
                    nc.vector.reciprocal_approx_fast(rinv[:, :], ps_rr[:, :])
                    osb = posb.tile([128, TCH], HDT, tag="osb", name=f"osb_{h}_{j}")
                    nc.vector.tensor_mul(osb[:, :], ps_o[:, :], rinv[:, :])
                    osbs[(h, j)] = osb

                pending["tail"] = [
                    (lambda idx=idx: consume(idx, idx))
                    for idx in range(max(0, L - SKEW), L)
                ]
                pending["fin"] = finalize

            # ---------------- main fused loop ----------------
            dma_chunk(0)
            phaseA_chunk0()
            for j in range(NCHB):
                if j == 0:
                    filler.append(dma_wo)     # Wo load off the critical preload
                if j + 1 < NCHB:
                    nxt = phaseA_thunks(j + 1)
                    filler.append(nxt[0])     # DMA thunk first
                    nxt = nxt[1:]
                else:
                    nxt = []
                prj = proj_thunks(j - 1) if j >= 1 else []
                # interleave the two filler streams
                inter = []
                na, nb = len(nxt), len(prj)
                ia = ib = 0
                for s in range(na + nb):
                    if ia * max(nb, 1) <= ib * max(na, 1) and ia < na:
                        inter.append(nxt[ia]); ia += 1
                    elif ib < nb:
                        inter.append(prj[ib]); ib += 1
                    else:
                        inter.append(nxt[ia]); ia += 1
                filler.extend(inter)
                for h in range(HPC):
                    attention(h, j)
                    if j == NCHB - 1:
                        # reserve fillers for after the last instance, where
                        # the final softmax chains otherwise expose
                        pop_filler(max(1, len(filler) // (2 * (HPC - h))))
                    else:
                        pop_filler(max(1, len(filler) // (HPC - h)))
                pop_filler(len(filler))
            flush_pending()
            for th in proj_thunks(NCHB - 1):
                th()

    nc.compile()
    return nc


def _prep_inputs(x, mask, pos, Wqkv, Wo):
    x = np.asarray(x, dtype=np.float32)
    pos = np.asarray(pos)
    inv = (
        np.float32(1.0)
        / (np.float32(THETA) ** (np.arange(0, DK, 2, dtype=np.float32) / np.float32(DK)))
    ).astype(np.float32)
    ang = pos.astype(np.float32)[:, None] * inv[None, :]  # [T, 64]
    cosT = np.cos(ang).astype(np.float32).T  # [64, T]
    sinT = np.sin(ang).astype(np.float32).T
    cs2 = np.ascontiguousarray(np.concatenate([cosT, cosT], 0), dtype=np.float32)
    # halves swapped: rows 0:64 multiply q1 (-> +sin), rows 64:128 multiply
    # q2 (-> -sin), each read at the same partition base as its q/k input
    sn2 = np.ascontiguousarray(np.concatenate([sinT, -sinT], 0), dtype=np.float32)
    # keep tk <= tq within the 128-col causal boundary window
    mp = np.ascontiguousarray(np.triu(np.ones((128, 128), dtype=np.float16)))

    Wqkv = np.asarray(Wqkv, dtype=np.float32)
    Wo = np.asarray(Wo, dtype=np.float32)
    xT = [
        np.ascontiguousarray(x[b].T).astype(np.float16) for b in range(B)
    ]
    in_maps = []
    for g in range(NCORES):
        b, hg = g // HG, g % HG
        c0 = hg * DL
        wqkv_g = np.concatenate(
            [Wqkv[:, c0 : c0 + DL], Wqkv[:, D + c0 : D + c0 + DL],
             Wqkv[:, 2 * D + c0 : 2 * D + c0 + DL]], axis=1
        ).astype(np.float16)
        wo_g = Wo[c0 : c0 + DL, :].astype(np.float16)
        in_maps.append(
            {"xt": xT[b], "wqkv": wqkv_g, "wo": wo_g, "cs2": cs2, "sn2": sn2,
             "mp": mp}
        )
    return in_maps


def _get_nc():
    if "nc" not in _cache:
        _cache["nc"] = _build()
    return _cache["nc"]


def run(x, mask, pos, Wqkv, Wo, trace=False):
    in_maps = _prep_inputs(x, mask, pos, Wqkv, Wo)
    nc = _get_nc()
    res = run_bass_kernel_spmd(nc, in_maps, core_ids=list(range(NCORES)), trace=trace)
    out = np.zeros((B, T, D), dtype=np.float64)
    for g, r in enumerate(res.results):
        out[g // HG] += r["out"].astype(np.float64)
    return out.astype(np.float32), res


def kernel(x, mask, pos, Wqkv, Wo):
    out, _ = run(x, mask, pos, Wqkv, Wo, trace=False)
    return out


# revision 52
# speedup vs baseline: 1.1894x; 1.1894x over previous
"""TRN2 Bass kernel for nn_Attention_75935021793702.

Dense transformer attention block:
    qkv = x @ Wqkv ; q,k = RoPE(q,k,pos) ; y = softmax(causal(q k^T / sqrt(dk))) v ; out = y @ Wo

Sharding: hybrid 2 (batch) x 4 (head-group) over 8 cores.  Each core handles
one batch and 4 heads: its slice of the QKV projection (columns of Wqkv), the
attention for its 4 heads, and a partial output projection (rows of Wo).  The
host sums 4 partials per batch.

All 16-bit operands are fp16 (better mantissa than bf16 at these magnitudes;
matmuls run at full PE rate).  PSUM accumulation is fp32.

Device dataflow (per core), fully fused pipeline over 4 token chunks j:
  phaseA(j): stream x^T chunk -> q^T,k^T (feature-major + RoPE on DVE) and v
             (token-major); q/k/v stay RESIDENT in SBUF (no DRAM spill).
  attn(h,j): S^T = k^T.T q^T on PE; exp on ACT (scale=1/sqrt(dk)) into an
             es strip [128, L, 512]; causal boundary via one shared
             [128,128] multiplicative 0/1 triangle on a 128-col window
             post-exp (gpsimd); A@V accumulates O^T on PE.  Softmax
             denominator: fp16 halving-tree partial sum on DVE down to <=4
             slots, then accumulating 128-wide all-ones matmuls on PE whose
             output replicates the cross-partition sum into every partition
             (no broadcast needed); on the last chunk the whole rowsum runs
             as per-tile all-ones matmuls on the then-idle PE/pap bank.
             reciprocal + O^T scale on DVE.
  proj(j):   out[tq,:] += sum_h O_h^T.T @ Wo_h, PSUM -> SBUF -> DRAM.
  phaseA(j+1) and proj(j-1) are emitted as FILLER work between (and inside)
  the attention instances of chunk j so the PE queue never drains.
"""

import sys

sys.path.insert(0, "/opt/trn_rl_repo")

import numpy as np
import concourse.bass as bass
import concourse.mybir as mybir
import concourse.tile as tile
from concourse import bacc
from concourse import bass_isa
from concourse.bass_utils import run_bass_kernel_spmd

F32 = mybir.dt.float32
F16 = mybir.dt.float16
EXP = mybir.ActivationFunctionType.Exp

HDT = F16             # dtype of all 16-bit matmul operands
NEG = -1.0e9          # additive causal-mask value (pre-exp)

B, T, D, H = 2, 2048, 2048, 16
DK = D // H                       # 128
THETA = 10000.0
NCORES = 8
HG = 4                            # head groups (cores per batch)
HPC = H // HG                     # heads per core = 4
DL = HPC * DK                     # local width = 512
TCH = 512                         # token chunk (matmul moving dim)
NCHB = T // TCH                   # 4 chunks per batch
KT = D // 128                     # 16 contraction tiles
NTT = T // 128                    # 16 tk tiles
SCALE = 1.0 / float(np.sqrt(np.float32(DK)))

_cache = {}


def _mm(nc, out, lhsT, rhs, start, stop):
    nc.tensor.matmul(out, lhsT, rhs, start=start, stop=stop)


def _build():
    nc = bacc.Bacc("TRN2", target_bir_lowering=False, debug=False)

    xt_d = nc.dram_tensor("xt", [D, T], HDT, kind="ExternalInput").ap()
    wqkv_d = nc.dram_tensor("wqkv", [D, 3 * DL], HDT, kind="ExternalInput").ap()
    wo_d = nc.dram_tensor("wo", [DL, D], HDT, kind="ExternalInput").ap()
    cs2_d = nc.dram_tensor("cs2", [DK, T], F32, kind="ExternalInput").ap()
    sn2_d = nc.dram_tensor("sn2", [DK, T], F32, kind="ExternalInput").ap()
    mp_d = nc.dram_tensor("mp", [128, 128], HDT, kind="ExternalInput").ap()
    out_d = nc.dram_tensor("out", [T, D], HDT, kind="ExternalOutput").ap()

    with tile.TileContext(nc) as tc:
        with (
            tc.tile_pool(name="const", bufs=1) as pc,
            tc.tile_pool(name="pax", bufs=20) as pax,      # x^T stream
            tc.tile_pool(name="pq", bufs=8) as pq,         # q^T chunk tiles
            tc.tile_pool(name="pat", bufs=3) as pat,       # RoPE temporaries
            tc.tile_pool(name="pes", bufs=2) as pes,       # exp(S) strips
            tc.tile_pool(name="prb", bufs=3) as prb,       # rowsum bcast/recip
            tc.tile_pool(name="posb", bufs=8) as posb,     # normalized O^T
            tc.tile_pool(name="pcs", bufs=3) as pcs,       # out staging
            tc.tile_pool(name="pap", bufs=2, space="PSUM") as pap,   # phase A
            tc.tile_pool(name="pbs", bufs=2, space="PSUM") as pbs,   # S
            tc.tile_pool(name="pbp", bufs=2, space="PSUM") as pbp,   # O acc
            tc.tile_pool(name="pcp", bufs=2, space="PSUM") as pcp,   # proj
        ):
            # ---------------- constants / resident tensors ----------------
            wq = pc.tile([128, KT, 3 * DL], HDT)
            cs2 = pc.tile([128, T], F32)
            sn2 = pc.tile([128, T], F32)
            mp = pc.tile([128, 128], HDT)   # 0/1 upper triangle (tk <= tq)
            wo = pc.tile([128, HPC, D], HDT)
            k_res = pc.tile([128, HPC, T], HDT)     # k^T feature-major
            v_res = pc.tile([128, NTT, DL], HDT)    # v token-major
            ones = pc.tile([128, 128], HDT)         # rowsum+broadcast matmul
            # memset on gpsimd: its queue is empty at t=0, unlike the DVE
            # which sits behind the framework's preamble const loads
            nc.gpsimd.memset(ones[:, :], 1.0)
            # dependency-free junk matmuls spanning the whole DMA lead-in:
            # trips the HAM activity window early AND keeps it warm (it
            # re-throttles after ~3.4us idle), so the PE clock is already
            # 2.4 GHz when the first real matmul issues at ~13us
            # warm lives in the proj PSUM pool, idle until j=1: junk can keep
            # firing between chunk-0 ki-rounds without touching the pools the
            # real chunk-0 groups rotate through
            warm = pcp.tile([128, TCH], F32, tag="pso", name="warm")
            for w in range(140):
                _mm(nc, warm[0:32, 0:32], ones[:, 0:32], ones[:, 0:32],
                    w == 0, w == 139)

            def junk(n):
                for w in range(n):
                    _mm(nc, warm[0:32, 0:32], ones[:, 0:32], ones[:, 0:32],
                        w == 0, w == n - 1)

            xts = {}     # (j, ki) -> xt tile
            q_sb = {}    # (j, h) -> q^T tile
            osbs = {}    # (h, j) -> normalized O^T tile

            def dma_chunk(j):
                tc0 = j * TCH
                for ki in range(KT):
                    if j == 0:
                        # q/k columns only: the first six matmul groups gate
                        # on these, the v columns stream afterwards
                        nc.sync.dma_start(
                            wq[:, ki, 0 : 2 * DL],
                            wqkv_d[128 * ki : 128 * ki + 128, 0 : 2 * DL],
                        )
                    xt = pax.tile([128, TCH], HDT, tag="xt", name=f"xt_{j}_{ki}")
                    nc.sync.dma_start(
                        xt[:, :], xt_d[128 * ki : 128 * ki + 128, tc0 : tc0 + TCH]
                    )
                    xts[(j, ki)] = xt
                    if j == 0 and ki == 1:
                        # RoPE/mask constants must beat the first psqk group
                        nc.sync.dma_start(cs2[:, :], cs2_d[:, :])
                        nc.sync.dma_start(sn2[:, :], sn2_d[:, :])
                        nc.sync.dma_start(mp[:, :], mp_d[:, :])
                if j == 0:
                    for ki in range(KT):
                        nc.sync.dma_start(
                            wq[:, ki, 2 * DL : 3 * DL],
                            wqkv_d[128 * ki : 128 * ki + 128, 2 * DL : 3 * DL],
                        )

            def dma_wo():
                for dt in range(HPC):
                    nc.sync.dma_start(wo[:, dt, :], wo_d[128 * dt : 128 * dt + 128, :])

            def mm_qk(j, nt, psqk, ki):
                _mm(nc, psqk[:, :], wq[:, ki, 128 * nt : 128 * nt + 128],
                    xts[(j, ki)][:, :], ki == 0, ki == KT - 1)

            def mm_v(j, tt, psv, ki):
                _mm(nc, psv[:, :], xts[(j, ki)][:, 128 * tt : 128 * tt + 128],
                    wq[:, ki, 2 * DL : 3 * DL], ki == 0, ki == KT - 1)

            def fin_qk(j, nt, psqk):
                # Free the PSUM bank immediately with a fast ACT copy into the
                # destination tile, then RoPE in-place on DVE (the DVE is not
                # on the PE's PSUM-rotation critical path this way).
                tc0 = j * TCH
                if nt < 4:
                    qt = pq.tile([128, TCH], HDT, tag="qt", name=f"qt_{j}_{nt}")
                    q_sb[(j, nt)] = qt
                    dst = lambda p0, p1: qt[p0:p1, :]
                else:
                    kh = nt - 4
                    dst = lambda p0, p1: k_res[p0:p1, kh, tc0 : tc0 + TCH]
                nc.scalar.copy(dst(0, 128), psqk[:, :])
                # sn2 halves are host-swapped so both SBUF inputs share the
                # same base partition (BIR verifier constraint)
                t2 = pat.tile([128, TCH], F32, tag="t2", name=f"t2_{j}_{nt}")
                nc.vector.tensor_mul(t2[0:64, :], dst(64, 128), sn2[64:128, tc0 : tc0 + TCH])
                nc.vector.tensor_mul(t2[64:128, :], dst(0, 64), sn2[0:64, tc0 : tc0 + TCH])
                t1 = pat.tile([128, TCH], F32, tag="t1", name=f"t1_{j}_{nt}")
                nc.vector.tensor_mul(t1[:, :], dst(0, 128), cs2[:, tc0 : tc0 + TCH])
                nc.vector.tensor_add(dst(0, 128), t1[:, :], t2[:, :])

            def fin_v(j, tt, psv):
                nc.scalar.copy(v_res[:, 4 * j + tt, :], psv[:, :])

            def qk_thunk(j, nt):
                def th():
                    psqk = pap.tile([128, TCH], F32, tag="ps", name=f"psqk_{j}_{nt}")
                    for ki in range(KT):
                        mm_qk(j, nt, psqk, ki)
                    fin_qk(j, nt, psqk)
                return th

            def v_thunk(j, tt):
                def th():
                    psv = pap.tile([128, DL], F32, tag="ps", name=f"psv_{j}_{tt}")
                    for ki in range(KT):
                        mm_v(j, tt, psv, ki)
                    fin_v(j, tt, psv)
                return th

            def phaseA_thunks(j):
                ths = [lambda j=j: dma_chunk(j)]
                ths += [qk_thunk(j, nt) for nt in range(8)]
                ths += [v_thunk(j, tt) for tt in range(4)]
                return ths

            def phaseA_chunk0():
                # Chunk 0 runs while weights/x still stream from HBM: issue
                # matmuls ki-major across 6 concurrently-open PSUM banks
                # (borrowing the idle attention pools) so the PE consumes
                # each arriving DMA tile for 6 groups at once instead of
                # stalling per-group.
                groups_a = [("q", 0), ("k", 4), ("q", 1), ("k", 5), ("q", 2), ("k", 6)]
                groups_b = [("q", 3), ("k", 7), ("v", 0), ("v", 1), ("v", 2), ("v", 3)]
                pools = [pap, pap, pbs, pbs, pbp, pbp]
                tags = ["ps", "ps", "ps_s", "ps_s", "ps_o", "ps_o"]
                for gi, grp in enumerate([groups_a, groups_b]):
                    ps = []
                    for g, (kind, idx) in enumerate(grp):
                        ps.append(pools[g].tile([128, TCH], F32, tag=tags[g],
                                                name=f"ps0_{gi}_{g}"))
                    for ki in range(KT):
                        for g, (kind, idx) in enumerate(grp):
                            if kind == "v":
                                mm_v(0, idx, ps[g], ki)
                            else:
                                mm_qk(0, idx, ps[g], ki)
                        # fill DMA-wait bubbles in the ramp and keep the
                        # HAM activity window tripped
                        junk(2)
                    for g, (kind, idx) in enumerate(grp):
                        if kind == "v":
                            fin_v(0, idx, ps[g])
                        else:
                            fin_qk(0, idx, ps[g])

            def proj_thunks(j):
                # out[tq,:] += sum_h O_h[tq,dl] @ Wo_h[dl,:] for chunk j
                ths = []
                for a in range(TCH // 128):
                    for ec in range(D // TCH):
                        def th(a=a, ec=ec, j=j):
                            trow = TCH * j + 128 * a
                            pso = pcp.tile([128, TCH], F32, tag="pso",
                                           name=f"psoc_{j}_{a}_{ec}")
                            for h in range(HPC):
                                _mm(nc, pso[:, :],
                                    osbs[(h, j)][:, 128 * a : 128 * a + 128],
                                    wo[:, h, TCH * ec : TCH * ec + TCH],
                                    h == 0, h == HPC - 1)
                            outsb = pcs.tile([128, TCH], HDT, tag="outsb",
                                             name=f"outsb_{j}_{a}_{ec}")
                            # DVE while attention is live (the scalar engine
                            # is exp-critical there); the last chunk's
                            # projection runs in the exp-free tail, where
                            # alternating engines doubles copy throughput
                            if j == NCHB - 1 and (a + ec) % 2 == 0:
                                nc.scalar.copy(outsb[:, :], pso[:, :])
                            else:
                                nc.vector.tensor_copy(outsb[:, :], pso[:, :])
                            nc.sync.dma_start(
                                out_d[trow : trow + 128, TCH * ec : TCH * ec + TCH],
                                outsb[:, :],
                            )
                        ths.append(th)
                return ths

            pending = {"tail": [], "fin": None}
            filler = []

            def flush_pending():
                for fn in pending["tail"]:
                    fn()
                pending["tail"] = []
                if pending["fin"] is not None:
                    pending["fin"]()
                    pending["fin"] = None

            def pop_filler(n):
                for _ in range(min(n, len(filler))):
                    filler.pop(0)()

            def attention(h, j):
                qt = q_sb[(j, h)]
                L = 4 * j + 4           # live tk tiles 0..L-1
                # Last chunk: no phase-A fillers exist, the PE has slack and
                # the pap PSUM pool is idle -> do the softmax denominator as
                # per-tile accumulating all-ones matmuls instead of the DVE
                # tree (the DVE is the bottleneck there).
                pe_rowsum = j == NCHB - 1
                ps_o = pbp.tile([128, TCH], F32, tag="ps_o", name=f"pso_{h}_{j}")
                ps_r = (pap.tile([128, TCH], F32, tag="ps", name=f"psr_{h}_{j}")
                        if pe_rowsum else None)
                es = pes.tile([128, NTT, TCH], HDT, tag="es", name=f"es_{h}_{j}")
                SKEW = 2

                def consume(i, idx):
                    st = idx == 0
                    sp = idx == L - 1
                    sl = 128 * (i - 4 * j) if i >= 4 * j else 0
                    _mm(nc, ps_o[:, sl:TCH],
                        v_res[:, i, 128 * h : 128 * h + 128],
                        es[:, i, sl:TCH], st, sp)
                    if pe_rowsum:
                        _mm(nc, ps_r[:, sl:TCH], ones[:, :], es[:, i, sl:TCH], st, sp)

                prev_tail = pending["tail"]
                prev_fin = pending["fin"]
                fin_done = [prev_fin is None]

                for i in range(L):
                    diag = i >= 4 * j
                    sl = 128 * (i - 4 * j) if diag else 0
                    ps_s = pbs.tile([128, TCH], F32, tag="ps_s", name=f"pss_{h}_{j}_{i}")
                    _mm(nc, ps_s[:, sl:TCH],
                        k_res[:, h, 128 * i : 128 * i + 128],
                        qt[:, sl:TCH], True, True)
                    if diag and sl > 0 and not pe_rowsum:
                        nc.gpsimd.memset(es[:, i, 0:sl], 0.0)
                    nc.scalar.activation(es[:, i, sl:TCH], ps_s[:, sl:TCH], EXP, scale=SCALE)
                    if diag:
                        # causal boundary: multiplicative 0/1 triangle on the
                        # 128-col window, post-exp, on the otherwise-idle
                        # gpsimd engine (it cannot touch PSUM, es is SBUF)
                        nc.gpsimd.tensor_mul(
                            es[:, i, sl : sl + 128], es[:, i, sl : sl + 128], mp[:, :]
                        )
                    # drain the previous instance's deferred work, one step per
                    # S-matmul, so the PE never waits on freshly issued exps
                    if prev_tail:
                        prev_tail.pop(0)()
                    elif not fin_done[0]:
                        prev_fin()
                        fin_done[0] = True
                    elif i % 2 == 1:
                        pop_filler(1)
                    if i >= SKEW:
                        consume(i - SKEW, i - SKEW)
                while prev_tail:
                    prev_tail.pop(0)()
                if not fin_done[0]:
                    prev_fin()
                    fin_done[0] = True

                def finalize():
                    if pe_rowsum:
                        ps_rr = ps_r
                    else:
                        # denominator: in-place fp16 halving-tree sum down to
                        # <=4 slots on DVE, then accumulating 128-wide
                        # all-ones matmuls on PE that sum across partitions
                        # AND replicate the result into every output
                        # partition (no broadcast needed)
                        lc = L
                        while lc > 4:
                            half = lc // 2
                            nc.vector.tensor_add(
                                es[:, 0:half, :], es[:, 0:half, :],
                                es[:, lc - half : lc, :]
                            )
                            lc -= half
                        ps_rr = pbs.tile([128, TCH], F32, tag="ps_s", name=f"psr_{h}_{j}")
                        for g in range(lc):
                            _mm(nc, ps_rr[:, :], ones[:, :], es[:, g, :],
                                g == 0, g == lc - 1)
                    rinv = prb.tile([128, TCH], F32, tag="rinv", name=f"rinv_{h}_{j}")
                    nc.vector.reciprocal_approx_fast(rinv[:, :], ps_rr[:, :])
                    osb = posb.tile([128, TCH], HDT, tag="osb", name=f"osb_{h}_{j}")
                    nc.vector.tensor_mul(osb[:, :], ps_o[:, :], rinv[:, :])
                    osbs[(h, j)] = osb

                pending["tail"] = [
                    (lambda idx=idx: consume(idx, idx))
                    for idx in range(max(0, L - SKEW), L)
                ]
                pending["fin"] = finalize

            # ---------------- main fused loop ----------------
            dma_chunk(0)
            phaseA_chunk0()
            for j in range(NCHB):
                if j == 0:
                    filler.append(dma_wo)     # Wo load off the critical preload
                if j + 1 < NCHB:
                    nxt = phaseA_thunks(j + 1)
                    filler.append(nxt[0])     # DMA thunk first
                    nxt = nxt[1:]
                else:
                    nxt = []
                prj = proj_thunks(j - 1) if j >= 1 else []
                # interleave the two filler streams
                inter = []
                na, nb = len(nxt), len(prj)
                ia = ib = 0
                for s in range(na + nb):
                    if ia * max(nb, 1) <= ib * max(na, 1) and ia < na:
                        inter.append(nxt[ia]); ia += 1
                    elif ib < nb:
                        inter.append(prj[ib]); ib += 1
                    else:
                        inter.append(nxt[ia]); ia += 1
                filler.extend(inter)
                for h in range(HPC):
                    attention(h, j)
                    if j == NCHB - 1:
                        # reserve fillers for after the last instance, where
                        # the final softmax chains otherwise expose
                        pop_filler(max(1, len(filler) // (2 * (HPC - h))))
                    else:
                        pop_filler(max(1, len(filler) // (HPC - h)))
                pop_filler(len(filler))
            flush_pending()
            for th in proj_thunks(NCHB - 1):
                th()

    nc.compile()
    return nc


def _prep_inputs(x, mask, pos, Wqkv, Wo):
    x = np.asarray(x, dtype=np.float32)
    pos = np.asarray(pos)
    inv = (
        np.float32(1.0)
        / (np.float32(THETA) ** (np.arange(0, DK, 2, dtype=np.float32) / np.float32(DK)))
    ).astype(np.float32)
    ang = pos.astype(np.float32)[:, None] * inv[None, :]  # [T, 64]
    cosT = np.cos(ang).astype(np.float32).T  # [64, T]
    sinT = np.sin(ang).astype(np.float32).T
    cs2 = np.ascontiguousarray(np.concatenate([cosT, cosT], 0), dtype=np.float32)
    # halves swapped: rows 0:64 multiply q1 (-> +sin), rows 64:128 multiply
    # q2 (-> -sin), each read at the same partition base as its q/k input
    sn2 = np.ascontiguousarray(np.concatenate([sinT, -sinT], 0), dtype=np.float32)
    # keep tk <= tq within the 128-col causal boundary window
    mp = np.ascontiguousarray(np.triu(np.ones((128, 128), dtype=np.float16)))

    Wqkv = np.asarray(Wqkv, dtype=np.float32)
    Wo = np.asarray(Wo, dtype=np.float32)
    xT = [
        np.ascontiguousarray(x[b].T).astype(np.float16) for b in range(B)
    ]
    in_maps = []
    for g in range(NCORES):
        b, hg = g // HG, g % HG
        c0 = hg * DL
        wqkv_g = np.concatenate(
            [Wqkv[:, c0 : c0 + DL], Wqkv[:, D + c0 : D + c0 + DL],
             Wqkv[:, 2 * D + c0 : 2 * D + c0 + DL]], axis=1
        ).astype(np.float16)
        wo_g = Wo[c0 : c0 + DL, :].astype(np.float16)
        in_maps.append(
            {"xt": xT[b], "wqkv": wqkv_g, "wo": wo_g, "cs2": cs2, "sn2": sn2,
             "mp": mp}
        )
    return in_maps


def _get_nc():
    if "nc" not in _cache:
        _cache["nc"] = _build()
    return _cache["nc"]


def run(x, mask, pos, Wqkv, Wo, trace=False):
    in_maps = _prep_inputs(x, mask, pos, Wqkv, Wo)
    nc = _get_nc()
    res = run_bass_kernel_spmd(nc, in_maps, core_ids=list(range(NCORES)), trace=trace)
    out = np.zeros((B, T, D), dtype=np.float64)
    for g, r in enumerate(res.results):
        out[g // HG] += r["out"].astype(np.float64)
    return out.astype(np.float32), res


def kernel(x, mask, pos, Wqkv, Wo):
    out, _ = run(x, mask, pos, Wqkv, Wo, trace=False)
    return out
